# revision 20
# baseline (speedup 1.0000x reference)
"""ARIMAX (nn_ARIMAX_16432544875252) Trainium2 kernel, 8 NeuronCores, data-parallel.

Math: the reference is a linear time-recurrence. With d_t = o_t - o_{t-1}:
    d_t = sum_{j=1..8} w_ar[8-j] d_{t-j} + sum_k w_ex[k] u[:, t-8+k]   (t >= 9)
    o_t = y8 + sum_{s=9..t} d_s ;   o_{0..8} = y
so the output is a linear map of (y, u). The AR companion matrix for the
actual weights has a near-unit-circle complex eigen pair (rho ~ 1+6e-5) and
all other modes decay fast (|lam| <= 0.91). The u->o Toeplitz kernel
CumQ(n) therefore splits as
    CumQ(n) = K0 + Re(Gamma lam^n) + R(n),   |R(n)| ~ 1e-6 for n >= 138.
The kernel computes, per 128-wide output chunk c:
  - banded part (u blocks c-1, c, c+1 plus the y/init block): exact Toeplitz
    matmuls on TensorE (bf16 in, f32 PSUM accumulate)
  - below-band part: rank-3 modal accumulators P_k^{(c)} (constant + the two
    oscillator components), built by matmuls with host-precomputed weight
    matrices, then added via one outer-product matmul per output group.
All small matrices (G bands, W modal weights, F modal bases) are derived on
the host from the 16 weights only. Host-side input staging: shard rows
across 8 cores, pad u to 33x128 cols, transpose to time-major, cast bf16.
"""

import numpy as np
import ml_dtypes

import concourse.bass as bass
import concourse.mybir as mybir
import concourse.tile as tile
from concourse import bacc
from concourse.bass_utils import run_bass_kernel_spmd

# ---- problem constants ----
P_AR = 8
M_EX = 8
S_SEQ = 4096
BATCH = 16384
NCOLS_U = S_SEQ + M_EX          # 4104
S_PAD = 4224                    # 33 u-blocks of 128
N_UBLK = 33
N_BLK = N_UBLK + 1              # + y block (j=0)
N_CHUNK = 32
N_CORES = 8
R_CORE = BATCH // N_CORES       # 2048
RQ = 512                        # rows per quarter
N_Q = R_CORE // RQ              # 4
N_RTQ = RQ // 128               # 4 row-tiles per quarter
N_GRP = S_SEQ // 512            # 8 output groups of 512
MODAL_MIN_N = 138
BF16 = mybir.dt.bfloat16
F32 = mybir.dt.float32

# band window (t_start, width) per stationary block j (0 = y, j>=1 = u block j-1).
# A u-block b only reaches output chunk b-1 through the 8-col e-window corner
# (tau_local >= s_local + 120), so the first 120 columns of the 384-wide
# Toeplitz window are identically zero and are trimmed.
def _block_windows():
    wins = {0: (0, 256), 1: (0, 256)}
    for j in range(2, 32):
        wins[j] = (128 * (j - 2) + 120, 264)
    wins[32] = (128 * 30 + 120, 136)
    wins[33] = (128 * 31 + 120, 8)
    return wins

WINS = _block_windows()


# --------------------------------------------------------------------------
# host-side decomposition (float64)
# --------------------------------------------------------------------------
def build_decomposition(w):
    w = np.asarray(w, np.float64)
    w_ar, w_ex = w[:P_AR], w[P_AR:]
    a = w_ar[::-1].copy()
    NMAX = 9 + S_SEQ + 8

    h = np.zeros(NMAX)
    h[0] = 1.0
    for n in range(1, NMAX):
        kk = min(P_AR, n)
        h[n] = np.dot(a[:kk], h[n - 1 : n - 1 - kk : -1] if n - 1 - kk >= 0 else h[n - 1 :: -1])
    Q = np.zeros(NMAX)
    for n in range(1, NMAX):
        jmax = min(P_AR, n)
        js = np.arange(1, jmax + 1)
        Q[n] = np.dot(w_ex[P_AR - js], h[n - js])
    CumQ = np.cumsum(Q)
    H = np.cumsum(h)

    # homogeneous responses and y-column coefficients
    Phi = np.zeros((P_AR, S_SEQ))
    for j in range(1, P_AR + 1):
        d = np.zeros(9 + S_SEQ + 1)
        d[j] = 1.0
        for s in range(9, 9 + S_SEQ):
            d[s] = np.dot(a, d[s - 1 : s - 9 : -1])
        Phi[j - 1] = d[9 : 9 + S_SEQ]
    Psi = np.cumsum(Phi, axis=1)
    C = np.zeros((9, S_SEQ))
    C[8] = 1.0
    for j in range(1, 9):
        C[j] += Psi[j - 1]
        C[j - 1] -= Psi[j - 1]

    # marginal eigen pair
    A = np.zeros((P_AR, P_AR))
    A[0] = a
    A[1:, :-1] = np.eye(P_AR - 1)
    eig = np.linalg.eigvals(A)
    cands = eig[np.abs(np.abs(eig) - np.abs(eig).max()) < 1e-9]
    pos = cands[np.imag(cands) > 0]
    lam = pos[0] if len(pos) else cands[0]

    n_fit = np.arange(MODAL_MIN_N, NMAX)
    lp = lam ** n_fit.astype(np.float64)
    Bm = np.stack([np.ones(len(n_fit)), lp.real, lp.imag], 1)
    coef, *_ = np.linalg.lstsq(Bm, CumQ[n_fit], rcond=None)
    K0 = coef[0]
    Gamma = coef[1] - 1j * coef[2]
    if abs(Gamma) < 1e-12:
        raise RuntimeError("degenerate modal fit")
    fit_err = np.abs(CumQ[n_fit] - Bm @ coef).max()

    tau_fit = np.arange(256, S_SEQ)
    lpy = lam ** (9 + tau_fit.astype(np.float64))
    By = np.stack([np.ones(len(tau_fit)), lpy.real, lpy.imag], 1)
    Kc = np.zeros(9)
    Gc = np.zeros(9, complex)
    for j in range(9):
        cj, *_ = np.linalg.lstsq(By, C[j, tau_fit], rcond=None)
        Kc[j], Gc[j] = cj[0], cj[1] - 1j * cj[2]

    # exact Theta columns for u cols s=1..7 (e_t only defined for t>=9)
    tau_all = np.arange(S_SEQ)
    Theta_small = np.zeros((8, S_SEQ))
    for s in range(1, 8):
        acc = np.zeros(S_SEQ)
        for tp in range(max(9, s + 1), s + 9):
            m = 9 + tau_all - tp
            acc += w_ex[s + 8 - tp] * np.where((m >= 0) & (m < NMAX), H[np.clip(m, 0, NMAX - 1)], 0.0)
        Theta_small[s] = acc
    Ks_small = np.zeros(8)
    Gs_small = np.zeros(8, complex)
    for s in range(1, 8):
        cj, *_ = np.linalg.lstsq(By, Theta_small[s, tau_fit], rcond=None)
        Ks_small[s], Gs_small[s] = cj[0], cj[1] - 1j * cj[2]

    # ---- G band matrices ----
    sl = np.arange(128)
    tw = np.arange(384)
    n_mid = tw[None, :] - sl[:, None] - 119
    G_full = np.where(n_mid >= 1, CumQ[np.clip(n_mid, 0, NMAX - 1)], 0.0)
    G_mid = G_full[:, 120:384]        # windows trimmed by 120 (see WINS)
    tau0 = np.arange(256)
    n0 = 9 + tau0[None, :] - sl[:, None]
    G_0 = np.where((n0 >= 1) & (sl[:, None] >= 8), CumQ[np.clip(n0, 0, NMAX - 1)], 0.0)
    G_0[:8, :] = Theta_small[:, :256]
    G_31 = G_full[:, 120:256]
    G_32 = G_full[:, 120:128]
    G_y = np.zeros((128, 256))
    G_y[:9] = C[:, :256]

    # ---- W modal weight matrices (rank slot layout: row 3c+k) ----
    s_all = np.arange(S_PAD)
    lam_neg = lam ** (-s_all.astype(np.float64))
    W = np.zeros((N_BLK, 128, 128))
    for b in range(N_UBLK):
        j = b + 1
        sg = 128 * b + sl
        g0 = np.where(sg >= 1, K0, 0.0)
        g1 = np.where(sg >= 1, lam_neg[sg].real, 0.0)
        g2 = np.where(sg >= 1, lam_neg[sg].imag, 0.0)
        if b == 0:
            zs = Gs_small / Gamma
            g0[:8] = np.where(sl[:8] >= 1, Ks_small, 0.0)
            g1[:8] = np.where(sl[:8] >= 1, zs.real, 0.0)
            g2[:8] = np.where(sl[:8] >= 1, zs.imag, 0.0)
        gk = np.stack([g0, g1, g2], 1)
        for c in range(b + 2, N_CHUNK):
            W[j, :, 3 * c : 3 * c + 3] = gk
    zy = Gc / Gamma
    for c in range(2, N_CHUNK):
        W[0, :9, 3 * c + 0] = Kc
        W[0, :9, 3 * c + 1] = zy.real
        W[0, :9, 3 * c + 2] = zy.imag

    # ---- F modal basis matrix [128 rank slots, 4096] ----
    lam_pos = lam ** (9 + tau_all.astype(np.float64))
    F1 = (Gamma * lam_pos).real
    F2 = -(Gamma * lam_pos).imag
    F = np.zeros((128, S_SEQ))
    for c in range(N_CHUNK):
        tsl = slice(128 * c, 128 * c + 128)
        F[3 * c + 0, tsl] = 1.0
        F[3 * c + 1, tsl] = F1[tsl]
        F[3 * c + 2, tsl] = F2[tsl]

    return dict(G_y=G_y, G_0=G_0, G_mid=G_mid, G_31=G_31, G_32=G_32,
                W=W, F=F, fit_err=fit_err)


# --------------------------------------------------------------------------
# bass graph (SPMD, one program for all 8 cores)
# --------------------------------------------------------------------------
def build_nc():
    nc = bacc.Bacc("TRN2", target_bir_lowering=False, debug=False,
                   enable_asserts=False, num_devices=N_CORES)
    # ut DRAM layout [128, N_Q, N_BLK, RQ]: per (partition, quarter) all 34
    # blocks' rows are contiguous -> large DMA runs
    ut_d = nc.dram_tensor("ut", [128, N_Q, N_BLK, RQ], BF16, kind="ExternalInput")
    gy_d = nc.dram_tensor("gy", [128, 256], BF16, kind="ExternalInput")
    g0_d = nc.dram_tensor("g0", [128, 256], BF16, kind="ExternalInput")
    gmid_d = nc.dram_tensor("gmid", [128, 264], BF16, kind="ExternalInput")
    g31_d = nc.dram_tensor("g31", [128, 136], BF16, kind="ExternalInput")
    g32_d = nc.dram_tensor("g32", [128, 8], BF16, kind="ExternalInput")
    wmat_d = nc.dram_tensor("wmat", [128, 31, 128], BF16, kind="ExternalInput")
    fmat_d = nc.dram_tensor("fmat", [128, S_SEQ], BF16, kind="ExternalInput")
    out_d = nc.dram_tensor("out", [R_CORE, S_SEQ], BF16, kind="ExternalOutput")

    # group -> contributing blocks
    contrib = {g: [] for g in range(N_GRP)}
    for j in range(N_BLK):
        t0, wd = WINS[j]
        for g in range(t0 // 512, (t0 + wd - 1) // 512 + 1):
            contrib[g].append(j)
    last_of = {g: max(js) for g, js in contrib.items()}

    with tile.TileContext(nc) as tc:
        with (
            tc.tile_pool(name="consts", bufs=1) as cpool,
            tc.tile_pool(name="ut", bufs=2) as upool,
            tc.tile_pool(name="pt", bufs=4) as ptpool,
            tc.tile_pool(name="outsb", bufs=2) as opool,
            tc.tile_pool(name="psum_pp", bufs=2, space="PSUM") as pppool,
            tc.tile_pool(name="psum", bufs=6, space="PSUM") as pspool,
        ):
            gy = cpool.tile([128, 256], BF16, tag="gy")
            g0 = cpool.tile([128, 256], BF16, tag="g0")
            gmid = cpool.tile([128, 264], BF16, tag="gmid")
            g31 = cpool.tile([128, 136], BF16, tag="g31")
            g32 = cpool.tile([128, 8], BF16, tag="g32")
            wmat = cpool.tile([128, 31 * 128], BF16, tag="wmat")
            fmat = cpool.tile([128, S_SEQ], BF16, tag="fmat")
            # wmat first: pass A needs it immediately (split so MM j=0 starts early)
            wmat3 = wmat.rearrange("p (j f) -> p j f", j=31)
            nc.sync.dma_start(out=wmat3[:, 0:4, :], in_=wmat_d[:, 0:4, :])
            nc.sync.dma_start(out=wmat3[:, 4:31, :], in_=wmat_d[:, 4:31, :])
            gsb = {0: gy, 1: g0, 32: g31, 33: g32}

            # DMA sub-chunk boundaries along the block axis
            JCHUNKS = [(0, 4), (4, 9), (9, 18), (18, 26), (26, N_BLK)]
            ut_tiles = []
            for q in range(N_Q):
                ut = upool.tile([128, N_BLK * RQ], BF16, tag="ut", name=f"ut_q{q}")
                ut_tiles.append(ut)
                for j0, j1 in JCHUNKS:
                    nc.sync.dma_start(
                        out=ut[:, j0 * RQ : j1 * RQ],
                        in_=ut_d[:, q, j0:j1, :],
                    )
                if q == 0:
                    # remaining consts: needed only once pass B starts
                    nc.sync.dma_start(out=gy[:], in_=gy_d[:])
                    nc.sync.dma_start(out=g0[:], in_=g0_d[:])
                    nc.sync.dma_start(out=gmid[:], in_=gmid_d[:])
                    nc.sync.dma_start(out=g31[:], in_=g31_d[:])
                    nc.sync.dma_start(out=g32[:], in_=g32_d[:])
                    nc.sync.dma_start(out=fmat[:], in_=fmat_d[:])

            for q in range(N_Q):
                ut = ut_tiles[q]
                # pass A: modal accumulators P^T for this quarter's 512 rows
                psum_p = pppool.tile([128, RQ], F32, tag="psum_p")
                for j in range(31):  # W is zero for j >= 31
                    nc.tensor.matmul(
                        psum_p[:],
                        lhsT=wmat[:, j * 128 : (j + 1) * 128],
                        rhs=ut[:, j * RQ : (j + 1) * RQ],
                        start=(j == 0),
                        stop=(j == 30),
                    )
                pt = ptpool.tile([128, RQ], BF16, tag="pt")
                nc.vector.tensor_copy(pt[:], psum_p[:])

                # pass B: banded matmuls + modal add, one row-tile at a time
                for rtq in range(N_RTQ):
                    r0 = rtq * 128
                    out_sb = opool.tile([128, S_SEQ], BF16, tag="outsb")
                    gpsum = {}
                    started = set()
                    for j in range(N_BLK):
                        t0, wd = WINS[j]
                        lhsT = ut[:, j * RQ + r0 : j * RQ + r0 + 128]
                        gmat = gsb.get(j, gmid)
                        for g in range(t0 // 512, (t0 + wd - 1) // 512 + 1):
                            glo, ghi = g * 512, g * 512 + 512
                            lo, hi = max(t0, glo), min(t0 + wd, ghi)
                            if g not in gpsum:
                                gpsum[g] = pspool.tile([128, 512], F32, tag="psum_g",
                                                       name=f"psg_q{q}r{rtq}g{g}")
                            nc.tensor.matmul(
                                gpsum[g][:, lo - glo : hi - glo],
                                lhsT=lhsT,
                                rhs=gmat[:, lo - t0 : hi - t0],
                                start=(g not in started),
                                stop=False,
                            )
                            started.add(g)
                            if j == last_of[g]:
                                # modal rank-3 add (chunks 0,1 have no modal part)
                                flo = glo if g > 0 else 256
                                nc.tensor.matmul(
                                    gpsum[g][:, flo - glo :],
                                    lhsT=pt[:, r0 : r0 + 128],
                                    rhs=fmat[:, flo:ghi],
                                    start=False,
                                    stop=True,
                                )
                                if g % 2 == 0:
                                    nc.vector.tensor_copy(out_sb[:, glo:ghi], gpsum[g][:])
                                else:
                                    nc.scalar.copy(out_sb[:, glo:ghi], gpsum[g][:])
                                del gpsum[g]
                                if g % 2 == 1:
                                    ho = (g - 1) * 512
                                    nc.sync.dma_start(
                                        out=out_d[q * RQ + r0 : q * RQ + r0 + 128,
                                                  ho : ho + 1024],
                                        in_=out_sb[:, ho : ho + 1024],
                                    )
    nc.compile()
    return nc


# --------------------------------------------------------------------------
# host staging + entry point
# --------------------------------------------------------------------------
def _prepare_in_maps(y, u, dec):
    y = np.asarray(y, np.float32)
    u = np.asarray(u, np.float32)
    consts = dict(
        gy=np.ascontiguousarray(dec["G_y"].astype(ml_dtypes.bfloat16)),
        g0=np.ascontiguousarray(dec["G_0"].astype(ml_dtypes.bfloat16)),
        gmid=np.ascontiguousarray(dec["G_mid"].astype(ml_dtypes.bfloat16)),
        g31=np.ascontiguousarray(dec["G_31"].astype(ml_dtypes.bfloat16)),
        g32=np.ascontiguousarray(dec["G_32"].astype(ml_dtypes.bfloat16)),
        wmat=np.ascontiguousarray(dec["W"][:31].transpose(1, 0, 2).astype(ml_dtypes.bfloat16)),
        fmat=np.ascontiguousarray(dec["F"].astype(ml_dtypes.bfloat16)),
    )
    in_maps = []
    for i in range(N_CORES):
        rows = slice(i * R_CORE, (i + 1) * R_CORE)
        ush = u[rows]
        ysh = y[rows]
        # layout [128, N_Q, N_BLK, RQ]: ut[p, q, 1+b, r] = u_pad[q*RQ+r, 128 b + p]
        ut = np.zeros((128, N_Q, N_BLK, RQ), ml_dtypes.bfloat16)
        ub = np.zeros((R_CORE, S_PAD), np.float32)
        ub[:, :NCOLS_U] = ush
        ut[:, :, 1:, :] = (
            ub.reshape(N_Q, RQ, N_UBLK, 128).transpose(3, 0, 2, 1).astype(ml_dtypes.bfloat16)
        )
        ut[:9, :, 0, :] = ysh.T.reshape(9, N_Q, RQ).astype(ml_dtypes.bfloat16)
        in_maps.append(dict(ut=np.ascontiguousarray(ut), **consts))
    return in_maps


def run_arimax(y, u, w, trace=False):
    dec = build_decomposition(w)
    in_maps = _prepare_in_maps(y, u, dec)
    nc = build_nc()
    res = run_bass_kernel_spmd(nc, in_maps, core_ids=list(range(N_CORES)), trace=trace)
    out = np.empty((BATCH, 9 + S_SEQ), np.float32)
    out[:, :9] = np.asarray(y, np.float32)
    for i in range(N_CORES):
        out[i * R_CORE : (i + 1) * R_CORE, 9:] = res.results[i]["out"].astype(np.float32)
    return out, res


def kernel(y, u, w):
    out, _ = run_arimax(y, u, w, trace=False)
    return out


# revision 21
# speedup vs baseline: 1.0028x; 1.0028x over previous
"""ARIMAX (nn_ARIMAX_16432544875252) Trainium2 kernel, 8 NeuronCores, data-parallel.

Math: the reference is a linear time-recurrence. With d_t = o_t - o_{t-1}:
    d_t = sum_{j=1..8} w_ar[8-j] d_{t-j} + sum_k w_ex[k] u[:, t-8+k]   (t >= 9)
    o_t = y8 + sum_{s=9..t} d_s ;   o_{0..8} = y
so the output is a linear map of (y, u). The AR companion matrix for the
actual weights has a near-unit-circle complex eigen pair (rho ~ 1+6e-5) and
all other modes decay fast (|lam| <= 0.91). The u->o Toeplitz kernel
CumQ(n) therefore splits as
    CumQ(n) = K0 + Re(Gamma lam^n) + R(n),   |R(n)| ~ 1e-6 for n >= 138.
The kernel computes, per 128-wide output chunk c:
  - banded part (u blocks c-1, c, c+1 plus the y/init block): exact Toeplitz
    matmuls on TensorE (bf16 in, f32 PSUM accumulate)
  - below-band part: rank-3 modal accumulators P_k^{(c)} (constant + the two
    oscillator components), built by matmuls with host-precomputed weight
    matrices, then added via one outer-product matmul per output group.
All small matrices (G bands, W modal weights, F modal bases) are derived on
the host from the 16 weights only. Host-side input staging: shard rows
across 8 cores, pad u to 33x128 cols, transpose to time-major, cast bf16.
"""

import numpy as np
import ml_dtypes

import concourse.bass as bass
import concourse.mybir as mybir
import concourse.tile as tile
from concourse import bacc
from concourse.bass_utils import run_bass_kernel_spmd

# ---- problem constants ----
P_AR = 8
M_EX = 8
S_SEQ = 4096
BATCH = 16384
NCOLS_U = S_SEQ + M_EX          # 4104
S_PAD = 4224                    # 33 u-blocks of 128
N_UBLK = 33
N_BLK = N_UBLK + 1              # + y block (j=0)
N_CHUNK = 32
N_CORES = 8
R_CORE = BATCH // N_CORES       # 2048
RQ = 512                        # rows per quarter
N_Q = R_CORE // RQ              # 4
N_RTQ = RQ // 128               # 4 row-tiles per quarter
N_GRP = S_SEQ // 512            # 8 output groups of 512
MODAL_MIN_N = 138
BF16 = mybir.dt.bfloat16
F32 = mybir.dt.float32

# band window (t_start, width) per stationary block j (0 = y, j>=1 = u block j-1).
# A u-block b only reaches output chunk b-1 through the 8-col e-window corner
# (tau_local >= s_local + 120), so the first 120 columns of the 384-wide
# Toeplitz window are identically zero and are trimmed.
def _block_windows():
    wins = {0: (0, 256), 1: (0, 256)}
    for j in range(2, 32):
        wins[j] = (128 * (j - 2) + 120, 264)
    wins[32] = (128 * 30 + 120, 136)
    wins[33] = (128 * 31 + 120, 8)
    return wins

WINS = _block_windows()


# --------------------------------------------------------------------------
# host-side decomposition (float64)
# --------------------------------------------------------------------------
def build_decomposition(w):
    w = np.asarray(w, np.float64)
    w_ar, w_ex = w[:P_AR], w[P_AR:]
    a = w_ar[::-1].copy()
    NMAX = 9 + S_SEQ + 8

    h = np.zeros(NMAX)
    h[0] = 1.0
    for n in range(1, NMAX):
        kk = min(P_AR, n)
        h[n] = np.dot(a[:kk], h[n - 1 : n - 1 - kk : -1] if n - 1 - kk >= 0 else h[n - 1 :: -1])
    Q = np.zeros(NMAX)
    for n in range(1, NMAX):
        jmax = min(P_AR, n)
        js = np.arange(1, jmax + 1)
        Q[n] = np.dot(w_ex[P_AR - js], h[n - js])
    CumQ = np.cumsum(Q)
    H = np.cumsum(h)

    # homogeneous responses and y-column coefficients
    Phi = np.zeros((P_AR, S_SEQ))
    for j in range(1, P_AR + 1):
        d = np.zeros(9 + S_SEQ + 1)
        d[j] = 1.0
        for s in range(9, 9 + S_SEQ):
            d[s] = np.dot(a, d[s - 1 : s - 9 : -1])
        Phi[j - 1] = d[9 : 9 + S_SEQ]
    Psi = np.cumsum(Phi, axis=1)
    C = np.zeros((9, S_SEQ))
    C[8] = 1.0
    for j in range(1, 9):
        C[j] += Psi[j - 1]
        C[j - 1] -= Psi[j - 1]

    # marginal eigen pair
    A = np.zeros((P_AR, P_AR))
    A[0] = a
    A[1:, :-1] = np.eye(P_AR - 1)
    eig = np.linalg.eigvals(A)
    cands = eig[np.abs(np.abs(eig) - np.abs(eig).max()) < 1e-9]
    pos = cands[np.imag(cands) > 0]
    lam = pos[0] if len(pos) else cands[0]

    n_fit = np.arange(MODAL_MIN_N, NMAX)
    lp = lam ** n_fit.astype(np.float64)
    Bm = np.stack([np.ones(len(n_fit)), lp.real, lp.imag], 1)
    coef, *_ = np.linalg.lstsq(Bm, CumQ[n_fit], rcond=None)
    K0 = coef[0]
    Gamma = coef[1] - 1j * coef[2]
    if abs(Gamma) < 1e-12:
        raise RuntimeError("degenerate modal fit")
    fit_err = np.abs(CumQ[n_fit] - Bm @ coef).max()

    tau_fit = np.arange(256, S_SEQ)
    lpy = lam ** (9 + tau_fit.astype(np.float64))
    By = np.stack([np.ones(len(tau_fit)), lpy.real, lpy.imag], 1)
    Kc = np.zeros(9)
    Gc = np.zeros(9, complex)
    for j in range(9):
        cj, *_ = np.linalg.lstsq(By, C[j, tau_fit], rcond=None)
        Kc[j], Gc[j] = cj[0], cj[1] - 1j * cj[2]

    # exact Theta columns for u cols s=1..7 (e_t only defined for t>=9)
    tau_all = np.arange(S_SEQ)
    Theta_small = np.zeros((8, S_SEQ))
    for s in range(1, 8):
        acc = np.zeros(S_SEQ)
        for tp in range(max(9, s + 1), s + 9):
            m = 9 + tau_all - tp
            acc += w_ex[s + 8 - tp] * np.where((m >= 0) & (m < NMAX), H[np.clip(m, 0, NMAX - 1)], 0.0)
        Theta_small[s] = acc
    Ks_small = np.zeros(8)
    Gs_small = np.zeros(8, complex)
    for s in range(1, 8):
        cj, *_ = np.linalg.lstsq(By, Theta_small[s, tau_fit], rcond=None)
        Ks_small[s], Gs_small[s] = cj[0], cj[1] - 1j * cj[2]

    # ---- G band matrices ----
    sl = np.arange(128)
    tw = np.arange(384)
    n_mid = tw[None, :] - sl[:, None] - 119
    G_full = np.where(n_mid >= 1, CumQ[np.clip(n_mid, 0, NMAX - 1)], 0.0)
    G_mid = G_full[:, 120:384]        # windows trimmed by 120 (see WINS)
    tau0 = np.arange(256)
    n0 = 9 + tau0[None, :] - sl[:, None]
    G_0 = np.where((n0 >= 1) & (sl[:, None] >= 8), CumQ[np.clip(n0, 0, NMAX - 1)], 0.0)
    G_0[:8, :] = Theta_small[:, :256]
    G_31 = G_full[:, 120:256]
    G_32 = G_full[:, 120:128]
    G_y = np.zeros((128, 256))
    G_y[:9] = C[:, :256]

    # ---- W modal weight matrices (rank slot layout: row 3c+k) ----
    s_all = np.arange(S_PAD)
    lam_neg = lam ** (-s_all.astype(np.float64))
    W = np.zeros((N_BLK, 128, 128))
    for b in range(N_UBLK):
        j = b + 1
        sg = 128 * b + sl
        g0 = np.where(sg >= 1, K0, 0.0)
        g1 = np.where(sg >= 1, lam_neg[sg].real, 0.0)
        g2 = np.where(sg >= 1, lam_neg[sg].imag, 0.0)
        if b == 0:
            zs = Gs_small / Gamma
            g0[:8] = np.where(sl[:8] >= 1, Ks_small, 0.0)
            g1[:8] = np.where(sl[:8] >= 1, zs.real, 0.0)
            g2[:8] = np.where(sl[:8] >= 1, zs.imag, 0.0)
        gk = np.stack([g0, g1, g2], 1)
        for c in range(b + 2, N_CHUNK):
            W[j, :, 3 * c : 3 * c + 3] = gk
    zy = Gc / Gamma
    for c in range(2, N_CHUNK):
        W[0, :9, 3 * c + 0] = Kc
        W[0, :9, 3 * c + 1] = zy.real
        W[0, :9, 3 * c + 2] = zy.imag

    # ---- F modal basis matrix [128 rank slots, 4096] ----
    lam_pos = lam ** (9 + tau_all.astype(np.float64))
    F1 = (Gamma * lam_pos).real
    F2 = -(Gamma * lam_pos).imag
    F = np.zeros((128, S_SEQ))
    for c in range(N_CHUNK):
        tsl = slice(128 * c, 128 * c + 128)
        F[3 * c + 0, tsl] = 1.0
        F[3 * c + 1, tsl] = F1[tsl]
        F[3 * c + 2, tsl] = F2[tsl]

    return dict(G_y=G_y, G_0=G_0, G_mid=G_mid, G_31=G_31, G_32=G_32,
                W=W, F=F, fit_err=fit_err)


# --------------------------------------------------------------------------
# bass graph (SPMD, one program for all 8 cores)
# --------------------------------------------------------------------------
def build_nc():
    nc = bacc.Bacc("TRN2", target_bir_lowering=False, debug=False,
                   enable_asserts=False, num_devices=N_CORES)
    # ut DRAM layout [128, N_Q, N_BLK, RQ]: per (partition, quarter) all 34
    # blocks' rows are contiguous -> large DMA runs
    ut_d = nc.dram_tensor("ut", [128, N_Q, N_BLK, RQ], BF16, kind="ExternalInput")
    gy_d = nc.dram_tensor("gy", [128, 256], BF16, kind="ExternalInput")
    g0_d = nc.dram_tensor("g0", [128, 256], BF16, kind="ExternalInput")
    gmid_d = nc.dram_tensor("gmid", [128, 264], BF16, kind="ExternalInput")
    g31_d = nc.dram_tensor("g31", [128, 136], BF16, kind="ExternalInput")
    g32_d = nc.dram_tensor("g32", [128, 8], BF16, kind="ExternalInput")
    wmat_d = nc.dram_tensor("wmat", [128, 31, 128], BF16, kind="ExternalInput")
    fmat_d = nc.dram_tensor("fmat", [128, S_SEQ], BF16, kind="ExternalInput")
    out_d = nc.dram_tensor("out", [R_CORE, S_SEQ], BF16, kind="ExternalOutput")

    # group -> contributing blocks
    contrib = {g: [] for g in range(N_GRP)}
    for j in range(N_BLK):
        t0, wd = WINS[j]
        for g in range(t0 // 512, (t0 + wd - 1) // 512 + 1):
            contrib[g].append(j)
    last_of = {g: max(js) for g, js in contrib.items()}

    with tile.TileContext(nc) as tc:
        with (
            tc.tile_pool(name="consts", bufs=1) as cpool,
            tc.tile_pool(name="ut", bufs=2) as upool,
            tc.tile_pool(name="pt", bufs=4) as ptpool,
            tc.tile_pool(name="outsb", bufs=2) as opool,
            tc.tile_pool(name="psum_pp", bufs=2, space="PSUM") as pppool,
            tc.tile_pool(name="psum", bufs=6, space="PSUM") as pspool,
        ):
            gy = cpool.tile([128, 256], BF16, tag="gy")
            g0 = cpool.tile([128, 256], BF16, tag="g0")
            gmid = cpool.tile([128, 264], BF16, tag="gmid")
            g31 = cpool.tile([128, 136], BF16, tag="g31")
            g32 = cpool.tile([128, 8], BF16, tag="g32")
            wmat = cpool.tile([128, 31 * 128], BF16, tag="wmat")
            fmat = cpool.tile([128, S_SEQ], BF16, tag="fmat")
            # wmat first: pass A needs it immediately (split so MM j=0 starts early)
            wmat3 = wmat.rearrange("p (j f) -> p j f", j=31)
            nc.sync.dma_start(out=wmat3[:, 0:4, :], in_=wmat_d[:, 0:4, :])
            nc.sync.dma_start(out=wmat3[:, 4:31, :], in_=wmat_d[:, 4:31, :])
            gsb = {0: gy, 1: g0, 32: g31, 33: g32}

            # DMA sub-chunk boundaries along the block axis
            JCHUNKS = [(0, 4), (4, 9), (9, 18), (18, 26), (26, N_BLK)]
            ut_tiles = []
            for q in range(N_Q):
                ut = upool.tile([128, N_BLK * RQ], BF16, tag="ut", name=f"ut_q{q}")
                ut_tiles.append(ut)
                for j0, j1 in JCHUNKS:
                    nc.sync.dma_start(
                        out=ut[:, j0 * RQ : j1 * RQ],
                        in_=ut_d[:, q, j0:j1, :],
                    )
                if q == 0:
                    # remaining consts: needed only once pass B starts
                    nc.sync.dma_start(out=gy[:], in_=gy_d[:])
                    nc.sync.dma_start(out=g0[:], in_=g0_d[:])
                    nc.sync.dma_start(out=gmid[:], in_=gmid_d[:])
                    nc.sync.dma_start(out=g31[:], in_=g31_d[:])
                    nc.sync.dma_start(out=g32[:], in_=g32_d[:])
                    nc.sync.dma_start(out=fmat[:], in_=fmat_d[:])

            for q in range(N_Q):
                ut = ut_tiles[q]
                # pass A: modal accumulators P^T for this quarter's 512 rows
                psum_p = pppool.tile([128, RQ], F32, tag="psum_p")
                for j in range(31):  # W is zero for j >= 31
                    nc.tensor.matmul(
                        psum_p[:],
                        lhsT=wmat[:, j * 128 : (j + 1) * 128],
                        rhs=ut[:, j * RQ : (j + 1) * RQ],
                        start=(j == 0),
                        stop=(j == 30),
                    )
                pt = ptpool.tile([128, RQ], BF16, tag="pt")
                nc.vector.tensor_copy(pt[:], psum_p[:])

                # pass B: banded matmuls + modal add, one row-tile at a time
                for rtq in range(N_RTQ):
                    r0 = rtq * 128
                    out_sb = opool.tile([128, S_SEQ], BF16, tag="outsb")
                    gpsum = {}
                    started = set()
                    for j in range(N_BLK):
                        t0, wd = WINS[j]
                        lhsT = ut[:, j * RQ + r0 : j * RQ + r0 + 128]
                        gmat = gsb.get(j, gmid)
                        for g in range(t0 // 512, (t0 + wd - 1) // 512 + 1):
                            glo, ghi = g * 512, g * 512 + 512
                            lo, hi = max(t0, glo), min(t0 + wd, ghi)
                            if g not in gpsum:
                                gpsum[g] = pspool.tile([128, 512], F32, tag="psum_g",
                                                       name=f"psg_q{q}r{rtq}g{g}")
                            nc.tensor.matmul(
                                gpsum[g][:, lo - glo : hi - glo],
                                lhsT=lhsT,
                                rhs=gmat[:, lo - t0 : hi - t0],
                                start=(g not in started),
                                stop=False,
                            )
                            started.add(g)
                            if j == last_of[g]:
                                # modal rank-3 add (chunks 0,1 have no modal part)
                                flo = glo if g > 0 else 256
                                nc.tensor.matmul(
                                    gpsum[g][:, flo - glo :],
                                    lhsT=pt[:, r0 : r0 + 128],
                                    rhs=fmat[:, flo:ghi],
                                    start=False,
                                    stop=True,
                                )
                                if g % 2 == 0:
                                    nc.vector.tensor_copy(out_sb[:, glo:ghi], gpsum[g][:])
                                else:
                                    nc.scalar.copy(out_sb[:, glo:ghi], gpsum[g][:])
                                del gpsum[g]
                                if g == 3 or g == 7:
                                    ho = (g - 3) * 512
                                    nc.sync.dma_start(
                                        out=out_d[q * RQ + r0 : q * RQ + r0 + 128,
                                                  ho : ho + 2048],
                                        in_=out_sb[:, ho : ho + 2048],
                                    )
    nc.compile()
    return nc


# --------------------------------------------------------------------------
# host staging + entry point
# --------------------------------------------------------------------------
def _prepare_in_maps(y, u, dec):
    y = np.asarray(y, np.float32)
    u = np.asarray(u, np.float32)
    consts = dict(
        gy=np.ascontiguousarray(dec["G_y"].astype(ml_dtypes.bfloat16)),
        g0=np.ascontiguousarray(dec["G_0"].astype(ml_dtypes.bfloat16)),
        gmid=np.ascontiguousarray(dec["G_mid"].astype(ml_dtypes.bfloat16)),
        g31=np.ascontiguousarray(dec["G_31"].astype(ml_dtypes.bfloat16)),
        g32=np.ascontiguousarray(dec["G_32"].astype(ml_dtypes.bfloat16)),
        wmat=np.ascontiguousarray(dec["W"][:31].transpose(1, 0, 2).astype(ml_dtypes.bfloat16)),
        fmat=np.ascontiguousarray(dec["F"].astype(ml_dtypes.bfloat16)),
    )
    in_maps = []
    for i in range(N_CORES):
        rows = slice(i * R_CORE, (i + 1) * R_CORE)
        ush = u[rows]
        ysh = y[rows]
        # layout [128, N_Q, N_BLK, RQ]: ut[p, q, 1+b, r] = u_pad[q*RQ+r, 128 b + p]
        ut = np.zeros((128, N_Q, N_BLK, RQ), ml_dtypes.bfloat16)
        ub = np.zeros((R_CORE, S_PAD), np.float32)
        ub[:, :NCOLS_U] = ush
        ut[:, :, 1:, :] = (
            ub.reshape(N_Q, RQ, N_UBLK, 128).transpose(3, 0, 2, 1).astype(ml_dtypes.bfloat16)
        )
        ut[:9, :, 0, :] = ysh.T.reshape(9, N_Q, RQ).astype(ml_dtypes.bfloat16)
        in_maps.append(dict(ut=np.ascontiguousarray(ut), **consts))
    return in_maps


def run_arimax(y, u, w, trace=False):
    dec = build_decomposition(w)
    in_maps = _prepare_in_maps(y, u, dec)
    nc = build_nc()
    res = run_bass_kernel_spmd(nc, in_maps, core_ids=list(range(N_CORES)), trace=trace)
    out = np.empty((BATCH, 9 + S_SEQ), np.float32)
    out[:, :9] = np.asarray(y, np.float32)
    for i in range(N_CORES):
        out[i * R_CORE : (i + 1) * R_CORE, 9:] = res.results[i]["out"].astype(np.float32)
    return out, res


def kernel(y, u, w):
    out, _ = run_arimax(y, u, w, trace=False)
    return out


# revision 22
# speedup vs baseline: 1.0520x; 1.0491x over previous
"""ARIMAX (nn_ARIMAX_16432544875252) Trainium2 kernel, 8 NeuronCores, data-parallel.

Math: the reference is a linear time-recurrence. With d_t = o_t - o_{t-1}:
    d_t = sum_{j=1..8} w_ar[8-j] d_{t-j} + sum_k w_ex[k] u[:, t-8+k]   (t >= 9)
    o_t = y8 + sum_{s=9..t} d_s ;   o_{0..8} = y
so the output is a linear map of (y, u). The AR companion matrix for the
actual weights has a near-unit-circle complex eigen pair (rho ~ 1+6e-5) and
all other modes decay fast (|lam| <= 0.91). The u->o Toeplitz kernel
CumQ(n) therefore splits as
    CumQ(n) = K0 + Re(Gamma lam^n) + R(n),   |R(n)| ~ 1e-6 for n >= 138.
The kernel computes, per 128-wide output chunk c:
  - banded part (u blocks c-1, c, c+1 plus the y/init block): exact Toeplitz
    matmuls on TensorE (bf16 in, f32 PSUM accumulate)
  - below-band part: rank-3 modal accumulators P_k^{(c)} (constant + the two
    oscillator components), built by matmuls with host-precomputed weight
    matrices, then added via one outer-product matmul per output group.
All small matrices (G bands, W modal weights, F modal bases) are derived on
the host from the 16 weights only. Host-side input staging: shard rows
across 8 cores, pad u to 33x128 cols, transpose to time-major, cast bf16.
"""

import numpy as np
import ml_dtypes

import concourse.bass as bass
import concourse.mybir as mybir
import concourse.tile as tile
from concourse import bacc
from concourse.bass_utils import run_bass_kernel_spmd

# ---- problem constants ----
P_AR = 8
M_EX = 8
S_SEQ = 4096
BATCH = 16384
NCOLS_U = S_SEQ + M_EX          # 4104
S_PAD = 4224                    # 33 u-blocks of 128
N_UBLK = 33
N_BLK = N_UBLK + 1              # + y block (j=0)
N_CHUNK = 32
N_CORES = 8
R_CORE = BATCH // N_CORES       # 2048
RQ = 512                        # rows per quarter
N_Q = R_CORE // RQ              # 4
N_RTQ = RQ // 128               # 4 row-tiles per quarter
N_GRP = S_SEQ // 512            # 8 output groups of 512
MODAL_MIN_N = 138
BF16 = mybir.dt.bfloat16
F32 = mybir.dt.float32

# band window (t_start, width) per stationary block j (0 = y, j>=1 = u block j-1).
# A u-block b only reaches output chunk b-1 through the 8-col e-window corner
# (tau_local >= s_local + 120), so the first 120 columns of the 384-wide
# Toeplitz window are identically zero and are trimmed.
def _block_windows():
    wins = {0: (0, 256), 1: (0, 256)}
    for j in range(2, 32):
        wins[j] = (128 * (j - 2) + 120, 264)
    wins[32] = (128 * 30 + 120, 136)
    wins[33] = (128 * 31 + 120, 8)
    return wins

WINS = _block_windows()


# --------------------------------------------------------------------------
# host-side decomposition (float64)
# --------------------------------------------------------------------------
def build_decomposition(w):
    w = np.asarray(w, np.float64)
    w_ar, w_ex = w[:P_AR], w[P_AR:]
    a = w_ar[::-1].copy()
    NMAX = 9 + S_SEQ + 8

    h = np.zeros(NMAX)
    h[0] = 1.0
    for n in range(1, NMAX):
        kk = min(P_AR, n)
        h[n] = np.dot(a[:kk], h[n - 1 : n - 1 - kk : -1] if n - 1 - kk >= 0 else h[n - 1 :: -1])
    Q = np.zeros(NMAX)
    for n in range(1, NMAX):
        jmax = min(P_AR, n)
        js = np.arange(1, jmax + 1)
        Q[n] = np.dot(w_ex[P_AR - js], h[n - js])
    CumQ = np.cumsum(Q)
    H = np.cumsum(h)

    # homogeneous responses and y-column coefficients
    Phi = np.zeros((P_AR, S_SEQ))
    for j in range(1, P_AR + 1):
        d = np.zeros(9 + S_SEQ + 1)
        d[j] = 1.0
        for s in range(9, 9 + S_SEQ):
            d[s] = np.dot(a, d[s - 1 : s - 9 : -1])
        Phi[j - 1] = d[9 : 9 + S_SEQ]
    Psi = np.cumsum(Phi, axis=1)
    C = np.zeros((9, S_SEQ))
    C[8] = 1.0
    for j in range(1, 9):
        C[j] += Psi[j - 1]
        C[j - 1] -= Psi[j - 1]

    # marginal eigen pair
    A = np.zeros((P_AR, P_AR))
    A[0] = a
    A[1:, :-1] = np.eye(P_AR - 1)
    eig = np.linalg.eigvals(A)
    cands = eig[np.abs(np.abs(eig) - np.abs(eig).max()) < 1e-9]
    pos = cands[np.imag(cands) > 0]
    lam = pos[0] if len(pos) else cands[0]

    n_fit = np.arange(MODAL_MIN_N, NMAX)
    lp = lam ** n_fit.astype(np.float64)
    Bm = np.stack([np.ones(len(n_fit)), lp.real, lp.imag], 1)
    coef, *_ = np.linalg.lstsq(Bm, CumQ[n_fit], rcond=None)
    K0 = coef[0]
    Gamma = coef[1] - 1j * coef[2]
    if abs(Gamma) < 1e-12:
        raise RuntimeError("degenerate modal fit")
    fit_err = np.abs(CumQ[n_fit] - Bm @ coef).max()

    tau_fit = np.arange(256, S_SEQ)
    lpy = lam ** (9 + tau_fit.astype(np.float64))
    By = np.stack([np.ones(len(tau_fit)), lpy.real, lpy.imag], 1)
    Kc = np.zeros(9)
    Gc = np.zeros(9, complex)
    for j in range(9):
        cj, *_ = np.linalg.lstsq(By, C[j, tau_fit], rcond=None)
        Kc[j], Gc[j] = cj[0], cj[1] - 1j * cj[2]

    # exact Theta columns for u cols s=1..7 (e_t only defined for t>=9)
    tau_all = np.arange(S_SEQ)
    Theta_small = np.zeros((8, S_SEQ))
    for s in range(1, 8):
        acc = np.zeros(S_SEQ)
        for tp in range(max(9, s + 1), s + 9):
            m = 9 + tau_all - tp
            acc += w_ex[s + 8 - tp] * np.where((m >= 0) & (m < NMAX), H[np.clip(m, 0, NMAX - 1)], 0.0)
        Theta_small[s] = acc
    Ks_small = np.zeros(8)
    Gs_small = np.zeros(8, complex)
    for s in range(1, 8):
        cj, *_ = np.linalg.lstsq(By, Theta_small[s, tau_fit], rcond=None)
        Ks_small[s], Gs_small[s] = cj[0], cj[1] - 1j * cj[2]

    # ---- G band matrices ----
    sl = np.arange(128)
    tw = np.arange(384)
    n_mid = tw[None, :] - sl[:, None] - 119
    G_full = np.where(n_mid >= 1, CumQ[np.clip(n_mid, 0, NMAX - 1)], 0.0)
    G_mid = G_full[:, 120:384]        # windows trimmed by 120 (see WINS)
    tau0 = np.arange(256)
    n0 = 9 + tau0[None, :] - sl[:, None]
    G_0 = np.where((n0 >= 1) & (sl[:, None] >= 8), CumQ[np.clip(n0, 0, NMAX - 1)], 0.0)
    G_0[:8, :] = Theta_small[:, :256]
    G_31 = G_full[:, 120:256]
    G_32 = G_full[:, 120:128]
    G_y = np.zeros((128, 256))
    G_y[:9] = C[:, :256]

    # ---- W modal weight matrices (rank slot layout: row 3c+k) ----
    s_all = np.arange(S_PAD)
    lam_neg = lam ** (-s_all.astype(np.float64))
    W = np.zeros((N_BLK, 128, 128))
    for b in range(N_UBLK):
        j = b + 1
        sg = 128 * b + sl
        g0 = np.where(sg >= 1, K0, 0.0)
        g1 = np.where(sg >= 1, lam_neg[sg].real, 0.0)
        g2 = np.where(sg >= 1, lam_neg[sg].imag, 0.0)
        if b == 0:
            zs = Gs_small / Gamma
            g0[:8] = np.where(sl[:8] >= 1, Ks_small, 0.0)
            g1[:8] = np.where(sl[:8] >= 1, zs.real, 0.0)
            g2[:8] = np.where(sl[:8] >= 1, zs.imag, 0.0)
        gk = np.stack([g0, g1, g2], 1)
        for c in range(b + 2, N_CHUNK):
            W[j, :, 3 * c : 3 * c + 3] = gk
    zy = Gc / Gamma
    for c in range(2, N_CHUNK):
        W[0, :9, 3 * c + 0] = Kc
        W[0, :9, 3 * c + 1] = zy.real
        W[0, :9, 3 * c + 2] = zy.imag

    # ---- F modal basis matrix [128 rank slots, 4096] ----
    lam_pos = lam ** (9 + tau_all.astype(np.float64))
    F1 = (Gamma * lam_pos).real
    F2 = -(Gamma * lam_pos).imag
    F = np.zeros((128, S_SEQ))
    for c in range(N_CHUNK):
        tsl = slice(128 * c, 128 * c + 128)
        F[3 * c + 0, tsl] = 1.0
        F[3 * c + 1, tsl] = F1[tsl]
        F[3 * c + 2, tsl] = F2[tsl]

    return dict(G_y=G_y, G_0=G_0, G_mid=G_mid, G_31=G_31, G_32=G_32,
                W=W, F=F, fit_err=fit_err)


# --------------------------------------------------------------------------
# bass graph (SPMD, one program for all 8 cores)
# --------------------------------------------------------------------------
def build_nc():
    nc = bacc.Bacc("TRN2", target_bir_lowering=False, debug=False,
                   enable_asserts=False, num_devices=N_CORES)
    # ut DRAM layout [128, N_Q, N_BLK, RQ]: per (partition, quarter) all 34
    # blocks' rows are contiguous -> large DMA runs
    ut_d = nc.dram_tensor("ut", [128, N_Q, N_BLK, RQ], BF16, kind="ExternalInput")
    gy_d = nc.dram_tensor("gy", [128, 256], BF16, kind="ExternalInput")
    g0_d = nc.dram_tensor("g0", [128, 256], BF16, kind="ExternalInput")
    gmid_d = nc.dram_tensor("gmid", [128, 264], BF16, kind="ExternalInput")
    g31_d = nc.dram_tensor("g31", [128, 136], BF16, kind="ExternalInput")
    g32_d = nc.dram_tensor("g32", [128, 8], BF16, kind="ExternalInput")
    wmat_d = nc.dram_tensor("wmat", [128, 31, 128], BF16, kind="ExternalInput")
    fmat_d = nc.dram_tensor("fmat", [128, S_SEQ], BF16, kind="ExternalInput")
    out_d = nc.dram_tensor("out", [R_CORE, S_SEQ], BF16, kind="ExternalOutput")

    # group -> contributing blocks
    contrib = {g: [] for g in range(N_GRP)}
    for j in range(N_BLK):
        t0, wd = WINS[j]
        for g in range(t0 // 512, (t0 + wd - 1) // 512 + 1):
            contrib[g].append(j)
    last_of = {g: max(js) for g, js in contrib.items()}

    with tile.TileContext(nc) as tc:
        with (
            tc.tile_pool(name="consts", bufs=1) as cpool,
            tc.tile_pool(name="ut", bufs=2) as upool,
            tc.tile_pool(name="pt", bufs=4) as ptpool,
            tc.tile_pool(name="outsb", bufs=2) as opool,
            tc.tile_pool(name="psum_pp", bufs=2, space="PSUM") as pppool,
            tc.tile_pool(name="psum", bufs=6, space="PSUM") as pspool,
        ):
            gy = cpool.tile([128, 256], BF16, tag="gy")
            g0 = cpool.tile([128, 256], BF16, tag="g0")
            gmid = cpool.tile([128, 264], BF16, tag="gmid")
            g31 = cpool.tile([128, 136], BF16, tag="g31")
            g32 = cpool.tile([128, 8], BF16, tag="g32")
            wmat = cpool.tile([128, 31 * 128], BF16, tag="wmat")
            fmat = cpool.tile([128, S_SEQ], BF16, tag="fmat")
            # wmat first: pass A needs it immediately
            nc.sync.dma_start(out=wmat.rearrange("p (j f) -> p j f", j=31), in_=wmat_d[:])
            gsb = {0: gy, 1: g0, 32: g31, 33: g32}

            # DMA sub-chunk boundaries along the block axis
            JCHUNKS = [(0, 9), (9, 18), (18, 26), (26, N_BLK)]
            ut_tiles = []
            for q in range(N_Q):
                ut = upool.tile([128, N_BLK * RQ], BF16, tag="ut", name=f"ut_q{q}")
                ut_tiles.append(ut)
                for j0, j1 in JCHUNKS:
                    nc.sync.dma_start(
                        out=ut[:, j0 * RQ : j1 * RQ],
                        in_=ut_d[:, q, j0:j1, :],
                    )
                if q == 0:
                    # remaining consts: needed only once pass B starts
                    nc.sync.dma_start(out=gy[:], in_=gy_d[:])
                    nc.sync.dma_start(out=g0[:], in_=g0_d[:])
                    nc.sync.dma_start(out=gmid[:], in_=gmid_d[:])
                    nc.sync.dma_start(out=g31[:], in_=g31_d[:])
                    nc.sync.dma_start(out=g32[:], in_=g32_d[:])
                    nc.sync.dma_start(out=fmat[:], in_=fmat_d[:])

            for q in range(N_Q):
                ut = ut_tiles[q]
                # pass A: modal accumulators P^T for this quarter's 512 rows
                psum_p = pppool.tile([128, RQ], F32, tag="psum_p")
                for j in range(31):  # W is zero for j >= 31
                    nc.tensor.matmul(
                        psum_p[:],
                        lhsT=wmat[:, j * 128 : (j + 1) * 128],
                        rhs=ut[:, j * RQ : (j + 1) * RQ],
                        start=(j == 0),
                        stop=(j == 30),
                    )
                pt = ptpool.tile([128, RQ], BF16, tag="pt")
                nc.vector.tensor_copy(pt[:], psum_p[:])

                # pass B: banded matmuls + modal add, one row-tile at a time
                for rtq in range(N_RTQ):
                    r0 = rtq * 128
                    out_sb = opool.tile([128, S_SEQ], BF16, tag="outsb")
                    gpsum = {}
                    started = set()
                    for j in range(N_BLK):
                        t0, wd = WINS[j]
                        lhsT = ut[:, j * RQ + r0 : j * RQ + r0 + 128]
                        gmat = gsb.get(j, gmid)
                        for g in range(t0 // 512, (t0 + wd - 1) // 512 + 1):
                            glo, ghi = g * 512, g * 512 + 512
                            lo, hi = max(t0, glo), min(t0 + wd, ghi)
                            if g not in gpsum:
                                gpsum[g] = pspool.tile([128, 512], F32, tag="psum_g",
                                                       name=f"psg_q{q}r{rtq}g{g}")
                            nc.tensor.matmul(
                                gpsum[g][:, lo - glo : hi - glo],
                                lhsT=lhsT,
                                rhs=gmat[:, lo - t0 : hi - t0],
                                start=(g not in started),
                                stop=False,
                            )
                            started.add(g)
                            if j == last_of[g]:
                                # modal rank-3 add (chunks 0,1 have no modal part)
                                flo = glo if g > 0 else 256
                                nc.tensor.matmul(
                                    gpsum[g][:, flo - glo :],
                                    lhsT=pt[:, r0 : r0 + 128],
                                    rhs=fmat[:, flo:ghi],
                                    start=False,
                                    stop=True,
                                )
                                if g % 2 == 0:
                                    nc.vector.tensor_copy(out_sb[:, glo:ghi], gpsum[g][:])
                                else:
                                    nc.scalar.copy(out_sb[:, glo:ghi], gpsum[g][:])
                                del gpsum[g]
                                if g == 3 or g == 7:
                                    ho = (g - 3) * 512
                                    nc.sync.dma_start(
                                        out=out_d[q * RQ + r0 : q * RQ + r0 + 128,
                                                  ho : ho + 2048],
                                        in_=out_sb[:, ho : ho + 2048],
                                    )
    nc.compile()
    return nc


# --------------------------------------------------------------------------
# host staging + entry point
# --------------------------------------------------------------------------
def _prepare_in_maps(y, u, dec):
    y = np.asarray(y, np.float32)
    u = np.asarray(u, np.float32)
    consts = dict(
        gy=np.ascontiguousarray(dec["G_y"].astype(ml_dtypes.bfloat16)),
        g0=np.ascontiguousarray(dec["G_0"].astype(ml_dtypes.bfloat16)),
        gmid=np.ascontiguousarray(dec["G_mid"].astype(ml_dtypes.bfloat16)),
        g31=np.ascontiguousarray(dec["G_31"].astype(ml_dtypes.bfloat16)),
        g32=np.ascontiguousarray(dec["G_32"].astype(ml_dtypes.bfloat16)),
        wmat=np.ascontiguousarray(dec["W"][:31].transpose(1, 0, 2).astype(ml_dtypes.bfloat16)),
        fmat=np.ascontiguousarray(dec["F"].astype(ml_dtypes.bfloat16)),
    )
    in_maps = []
    for i in range(N_CORES):
        rows = slice(i * R_CORE, (i + 1) * R_CORE)
        ush = u[rows]
        ysh = y[rows]
        # layout [128, N_Q, N_BLK, RQ]: ut[p, q, 1+b, r] = u_pad[q*RQ+r, 128 b + p]
        ut = np.zeros((128, N_Q, N_BLK, RQ), ml_dtypes.bfloat16)
        ub = np.zeros((R_CORE, S_PAD), np.float32)
        ub[:, :NCOLS_U] = ush
        ut[:, :, 1:, :] = (
            ub.reshape(N_Q, RQ, N_UBLK, 128).transpose(3, 0, 2, 1).astype(ml_dtypes.bfloat16)
        )
        ut[:9, :, 0, :] = ysh.T.reshape(9, N_Q, RQ).astype(ml_dtypes.bfloat16)
        in_maps.append(dict(ut=np.ascontiguousarray(ut), **consts))
    return in_maps


def run_arimax(y, u, w, trace=False):
    dec = build_decomposition(w)
    in_maps = _prepare_in_maps(y, u, dec)
    nc = build_nc()
    res = run_bass_kernel_spmd(nc, in_maps, core_ids=list(range(N_CORES)), trace=trace)
    out = np.empty((BATCH, 9 + S_SEQ), np.float32)
    out[:, :9] = np.asarray(y, np.float32)
    for i in range(N_CORES):
        out[i * R_CORE : (i + 1) * R_CORE, 9:] = res.results[i]["out"].astype(np.float32)
    return out, res


def kernel(y, u, w):
    out, _ = run_arimax(y, u, w, trace=False)
    return out
